# revision 1
# baseline (speedup 1.0000x reference)
"""MoE routed dynamics kernel for Trainium2 (8 NeuronCores, expert-parallel).

Problem: for each row b of a [B, D+A] input, route through one of P=8
two-layer MLPs selected by policy_indices[b]:
    h = relu(x @ W1[p] + b1[p]);  y = h @ W2[p] + b2[p]

Sharding: expert-parallel. Core p owns expert p's weights (resident in
SBUF) and processes exactly the rows routed to expert p. The all-to-all
dispatch keyed on policy_indices happens on the host at shard time
(gather rows by expert, pad to a common capacity C), and the inverse
scatter happens at unshard time.

Device kernel (per core), all activations kept feature-on-partition so
no transposes are needed anywhere:
    xT   [DA, C]  (DA=576)         input, transposed on host
    hT   [H, C]   = relu(W1.T @ x + b1), H=1024, via PE matmuls
    outT [D, C]   = W2.T @ h + b2,  D=512
Matmuls run as out[M,N] = lhsT.T @ rhs with lhsT = weight chunks in
their natural [K, M] layout and rhs = activation chunks [K, N<=512].

Matmul dtype is float32r end-to-end (DRAM params, SBUF tiles, and the
relu output): full fp32 operand bits, ~2 PE cycles/row streaming. The
walrus birverifier requires every producer feeding an FP32r matmul to
carry the float32r dtype. Set _MM_DTYPE = "bfloat16" for 1 cycle/row at
bf16 operand precision (host pre-casts inputs).
"""

import math

import numpy as np

_B = 16384
_P = 8
_D = 512
_A = 64
_H = 1024
_DA = _D + _A   # 576
_DAP = 640      # _DA zero-padded to 5*128: uniform K=128 matmuls (the
                # ragged K=64 tail matmul measurably breaks the PE's
                # LDWEIGHTS pipelining, ~0.3us per L1 group)
_N_CORES = 8

_MM_DTYPE = "float32r"

_kernel_cache: dict = {}


def _k_chunks(total: int, step: int = 128):
    return [(k0, min(step, total - k0)) for k0 in range(0, total, step)]


def _build_bass(C: int):
    import concourse.bacc as bacc
    import concourse.mybir as mybir
    from concourse.tile import TileContext

    fp32 = mybir.dt.float32
    mmdt = getattr(mybir.dt, _MM_DTYPE)
    act = mybir.ActivationFunctionType

    assert C % 256 == 0, C
    n_chunks = [(n0, min(512, C - n0)) for n0 in range(0, C, 512)]
    k1 = _k_chunks(_DAP)  # 5 uniform K=128 chunks over padded DA
    k2 = _k_chunks(_H)    # 8 chunks over H=1024
    mh = _H // 128        # 8 output tiles of layer 1
    md = _D // 128        # 4 output tiles of layer 2

    nc = bacc.Bacc()
    xT = nc.declare_dram_parameter("xT", [_DAP, C], mmdt, isOutput=False)
    w1 = nc.declare_dram_parameter("w1", [_DAP, _H], mmdt, isOutput=False)
    b1 = nc.declare_dram_parameter("b1", [128, mh], fp32, isOutput=False)
    w2 = nc.declare_dram_parameter("w2", [_H, _D], mmdt, isOutput=False)
    b2 = nc.declare_dram_parameter("b2", [128, md], fp32, isOutput=False)
    outT = nc.declare_dram_parameter("outT", [_D, C], fp32, isOutput=True)

    with TileContext(nc) as tc:
        with (
            tc.tile_pool(name="wpool", bufs=1) as wpool,
            tc.tile_pool(name="xpool", bufs=3) as xpool,
            tc.tile_pool(name="hpool", bufs=2) as hpool,
            tc.tile_pool(name="ypool", bufs=3) as ypool,
            tc.tile_pool(name="ps1", bufs=4, space="PSUM") as ps1,
            tc.tile_pool(name="ps2", bufs=4, space="PSUM") as ps2,
        ):
            def dma_x(n0, nl):
                tiles = []
                for k0, kl in k1:
                    t = xpool.tile([128, nl], mmdt, tag=f"x_{k0}")
                    nc.sync.dma_start(out=t[:kl, :], in_=xT[k0 : k0 + kl, n0 : n0 + nl])
                    tiles.append(t)
                return tiles

            # DMA issue on the Sync engine is serial (~0.6us each), so
            # issue order sets how soon the PE can start. Interleave the
            # chunk-0 x tiles with the first-needed halves of w1 (cols
            # 0:512 serve L1 groups m=0..3), then the rest; w2 (needed
            # only ~15us in) goes last.
            x_first = []
            w1_sb = []
            for i, (k0, kl) in enumerate(k1):
                n0, nl = n_chunks[0]
                xt = xpool.tile([128, nl], mmdt, tag=f"x_{k0}")
                nc.sync.dma_start(out=xt[:kl, :], in_=xT[k0 : k0 + kl, n0 : n0 + nl])
                x_first.append(xt)
                wt = wpool.tile([128, _H], mmdt, tag=f"w1_{k0}")
                nc.sync.dma_start(out=wt[:kl, : _H // 2], in_=w1[k0 : k0 + kl, : _H // 2])
                w1_sb.append(wt)
            # w1b_0 gates L1 group m=4's first matmul; b1 gates the whole
            # relu chain (and so PSUM recycling). Issue those right after
            # the interleave — the b vectors are 4KB and cost nothing —
            # instead of burying b1 behind 5 more 256KB transfers.
            nc.sync.dma_start(out=w1_sb[0][:, _H // 2 :], in_=w1[0:128, _H // 2 :])
            b1_sb = wpool.tile([128, mh], fp32, tag="b1")
            nc.sync.dma_start(out=b1_sb[:], in_=b1[:, :])
            b2_sb = wpool.tile([128, md], fp32, tag="b2")
            nc.sync.dma_start(out=b2_sb[:], in_=b2[:, :])
            for i, (k0, kl) in enumerate(k1):
                if i == 0:
                    continue
                nc.sync.dma_start(
                    out=w1_sb[i][:kl, _H // 2 :], in_=w1[k0 : k0 + kl, _H // 2 :]
                )
            w2_sb = []
            for k0, kl in k2:
                t = wpool.tile([128, _D], mmdt, tag=f"w2_{k0}")
                nc.sync.dma_start(out=t[:kl, :], in_=w2[k0 : k0 + kl, :])
                w2_sb.append(t)

            for ci, (n0, nl) in enumerate(n_chunks):
                x_sb = x_first if ci == 0 else dma_x(n0, nl)

                h_sb = []
                for m in range(mh):
                    ps = ps1.tile([128, nl], fp32, tag="ps1")
                    for i, (k0, kl) in enumerate(k1):
                        nc.tensor.matmul(
                            ps[:, :],
                            w1_sb[i][:kl, m * 128 : (m + 1) * 128],
                            x_sb[i][:kl, :],
                            start=(i == 0),
                            stop=(i == len(k1) - 1),
                        )
                    ht = hpool.tile([128, nl], mmdt, tag=f"h_{m}")
                    nc.scalar.activation(ht[:], ps[:], act.Relu, bias=b1_sb[:, m : m + 1])
                    h_sb.append(ht)

                for d in range(md):
                    ps = ps2.tile([128, nl], fp32, tag="ps2")
                    for m in range(mh):
                        nc.tensor.matmul(
                            ps[:, :],
                            w2_sb[m][:, d * 128 : (d + 1) * 128],
                            h_sb[m][:, :],
                            start=(m == 0),
                            stop=(m == mh - 1),
                        )
                    yt = ypool.tile([128, nl], fp32, tag="y")
                    # Bias-add on DVE (idle) instead of ACT (busy with
                    # relu), split in halves so the store of the first half
                    # overlaps the second (shrinks the kernel tail).
                    for h0 in range(0, nl, 256):
                        h1 = min(h0 + 256, nl)
                        nc.vector.tensor_scalar_add(
                            yt[:, h0:h1], ps[:, h0:h1], b2_sb[:, d : d + 1]
                        )
                        nc.sync.dma_start(
                            out=outT[d * 128 : (d + 1) * 128, n0 + h0 : n0 + h1],
                            in_=yt[:, h0:h1],
                        )

    nc.compile()
    return nc


def _get_bass(C: int):
    nc = _kernel_cache.get(C)
    if nc is None:
        nc = _build_bass(C)
        _kernel_cache[C] = nc
    return nc


def _mm_np(a):
    """Cast a float32 array to the numpy dtype matching _MM_DTYPE."""
    if _MM_DTYPE == "bfloat16":
        import ml_dtypes

        return np.ascontiguousarray(a.astype(ml_dtypes.bfloat16))
    return np.ascontiguousarray(a)


def _prepare_in_maps(latents, actions, policy_indices, W1, b1, W2, b2):
    """Expert-parallel dispatch: returns (in_maps, C, order, offs, counts)."""
    latents = np.asarray(latents, dtype=np.float32)
    actions = np.asarray(actions, dtype=np.float32)
    pi = np.asarray(policy_indices).astype(np.int64)
    W1 = np.asarray(W1, dtype=np.float32)
    b1 = np.asarray(b1, dtype=np.float32)
    W2 = np.asarray(W2, dtype=np.float32)
    b2 = np.asarray(b2, dtype=np.float32)

    B = latents.shape[0]
    counts = np.bincount(pi, minlength=_P)
    order = np.argsort(pi, kind="stable")
    offs = np.concatenate(([0], np.cumsum(counts)))

    # Common per-core capacity; multiple of 256 so every matmul free dim
    # is >= 256 and chunks are 512 with one optional 256 tail.
    C = max(2304, int(math.ceil(counts.max() / 256)) * 256)

    x = np.empty((B, _DA), dtype=np.float32)
    x[:, :_D] = latents
    x[:, _D:] = actions
    x_sorted = x[order]

    in_maps = []
    for p in range(_P):
        xp = np.zeros((_DAP, C), dtype=np.float32)
        xp[:_DA, : counts[p]] = x_sorted[offs[p] : offs[p + 1]].T
        w1p = np.zeros((_DAP, _H), dtype=np.float32)
        w1p[:_DA] = W1[p]
        in_maps.append(
            {
                "xT": _mm_np(xp),
                "w1": _mm_np(w1p),
                "b1": np.ascontiguousarray(b1[p].reshape(_H // 128, 128).T),
                "w2": _mm_np(W2[p]),
                "b2": np.ascontiguousarray(b2[p].reshape(_D // 128, 128).T),
            }
        )
    return in_maps, C, order, offs, counts


def kernel(latents, actions, policy_indices, W1, b1, W2, b2):
    from concourse.bass_utils import run_bass_kernel_spmd

    in_maps, C, order, offs, counts = _prepare_in_maps(
        latents, actions, policy_indices, W1, b1, W2, b2
    )
    nc = _get_bass(C)
    results = run_bass_kernel_spmd(nc, in_maps, list(range(_N_CORES))).results

    B = np.asarray(latents).shape[0]
    out = np.empty((B, _D), dtype=np.float32)
    for p in range(_P):
        yT = results[p]["outT"]
        out[order[offs[p] : offs[p + 1]]] = yT[:, : counts[p]].T
    return out



# revision 4
# speedup vs baseline: 1.0121x; 1.0121x over previous
"""MoE routed dynamics kernel for Trainium2 (8 NeuronCores, expert-parallel).

Problem: for each row b of a [B, D+A] input, route through one of P=8
two-layer MLPs selected by policy_indices[b]:
    h = relu(x @ W1[p] + b1[p]);  y = h @ W2[p] + b2[p]

Sharding: expert-parallel. Core p owns expert p's weights and processes
the rows routed to expert p. The all-to-all dispatch keyed on
policy_indices happens on the host at shard time (gather rows by expert,
pad to a common capacity C multiple of 128), and the inverse scatter at
unshard time.

v2 design notes (from trace analysis of the fp32r baseline, 110.2us):
- fp32r already streams 1 col/cycle at N>=256, so bf16 does NOT halve
  matmul time; it halves DMA bytes and LDWEIGHTS, and (crucially) lifts
  the moving-operand limit from 512 to 1024 columns, halving matmul
  instruction count. End-to-end bf16 rel err ~3.4e-3 (budget 2e-2).
- DMA issue on the Sync engine costs ~600ns each (81 issues = 51us
  serialized in the baseline). Host pre-arranges k-major layouts so each
  logical transfer is ONE DMA (13 total), split across sync/scalar
  (HWDGE) for the startup-critical loads and gpsimd (SWDGE, ~25ns
  dispatch) for the bulk.
- The PE clock (HAM) runs 1.2GHz for the first ~3.4us of activity;
  warmup matmuls on a zeroed tile during the initial DMA wait ramp it
  to 2.4GHz before real work arrives.
- Output is stored bf16 and upcast on the host (error ~0.4%, halves
  store bytes and the kernel tail).
- C is rounded to 128 (not 256): bf16 matmuls run full rate at any N.
"""

import math

import numpy as np
import ml_dtypes

_B = 16384
_P = 8
_D = 512
_A = 64
_H = 1024
_DA = _D + _A   # 576
_KC = 5         # K chunks over DA padded to 5*128=640
_N_CORES = 8

_BF16 = ml_dtypes.bfloat16

_kernel_cache: dict = {}


def _chunks(C: int):
    """Column chunking: 512-wide chunks (walrus ISA caps the matmul
    moving operand at 512 elements) with the sub-512 remainder last, so
    the kernel tail drains a small chunk."""
    assert C % 128 == 0 and C > 0, C
    out = [512] * (C // 512)
    if C % 512:
        out.append(C % 512)
    return out


def _build_bass(C: int):
    import concourse.bacc as bacc
    import concourse.mybir as mybir
    from concourse.tile import TileContext

    fp32 = mybir.dt.float32
    bf16 = mybir.dt.bfloat16
    act = mybir.ActivationFunctionType

    widths = _chunks(C)
    offsets = [sum(widths[:i]) for i in range(len(widths))]
    mh = _H // 128  # 8 L1 output groups
    md = _D // 128  # 4 L2 output groups

    nc = bacc.Bacc()
    xd = nc.declare_dram_parameter("xq", [128, _KC, C], bf16, isOutput=False)
    w1d = nc.declare_dram_parameter("w1q", [128, _KC, _H], bf16, isOutput=False)
    w2d = nc.declare_dram_parameter("w2q", [128, mh, _D], bf16, isOutput=False)
    bd = nc.declare_dram_parameter("bq", [128, mh + md], fp32, isOutput=False)
    od = nc.declare_dram_parameter("oq", [128, md, C], bf16, isOutput=True)

    with TileContext(nc) as tc:
        with (
            tc.tile_pool(name="wpool", bufs=1) as wpool,
            tc.tile_pool(name="xpool", bufs=len(widths)) as xpool,
            tc.tile_pool(name="hpool", bufs=2) as hpool,
            tc.tile_pool(name="ypool", bufs=2) as ypool,
            tc.tile_pool(name="ps1", bufs=4, space="PSUM") as ps1,
            tc.tile_pool(name="ps2", bufs=4, space="PSUM") as ps2,
        ):
            w1_sb = wpool.tile([128, _KC, _H], bf16, tag="w1")
            w2_sb = wpool.tile([128, mh, _D], bf16, tag="w2")
            b_sb = wpool.tile([128, mh + md], fp32, tag="b")
            warm_sb = wpool.tile([128, 640], bf16, tag="warm")

            x_sb = [
                xpool.tile([128, _KC, nl], bf16, tag="x", name=f"x{ci}")
                for ci, nl in enumerate(widths)
            ]

            # Startup-critical loads on the two HWDGE engines (run
            # concurrently with each other and with gpsimd SWDGE below).
            nc.sync.dma_start(out=x_sb[0][:, :, :], in_=xd[:, :, 0 : widths[0]])
            nc.scalar.dma_start(out=w1_sb[:, :, 0:256], in_=w1d[:, :, 0:256])
            nc.scalar.dma_start(out=b_sb[:, :], in_=bd[:, :])
            # Warmup source tile (zeros) for PE clock ramp.
            nc.vector.memset(warm_sb[:, :], 0.0)
            # Bulk loads on gpsimd (software DGE: ~25ns engine dispatch,
            # ~1us generation each on the otherwise idle Pool engine).
            nc.gpsimd.dma_start(out=w1_sb[:, :, 256:512], in_=w1d[:, :, 256:512])
            nc.gpsimd.dma_start(out=w1_sb[:, :, 512:_H], in_=w1d[:, :, 512:_H])
            nc.gpsimd.dma_start(out=w2_sb[:, :, :], in_=w2d[:, :, :])
            for ci in range(1, len(widths)):
                n0, nl = offsets[ci], widths[ci]
                nc.gpsimd.dma_start(
                    out=x_sb[ci][:, :, :], in_=xd[:, :, n0 : n0 + nl]
                )

            # PE warmup: the HAM clock gate holds the PE at 1.2GHz for
            # ~3.4us of activity; burn that during the x0/w1 DMA wait so
            # real matmuls start at 2.4GHz.
            for _ in range(6):
                wp = ps1.tile([128, 512], fp32, tag="ps1", name="warmps")
                nc.tensor.matmul(
                    wp[:, :], warm_sb[:, 0:128], warm_sb[:, 128:640],
                    start=True, stop=True,
                )

            for ci, nl in enumerate(widths):
                n0 = offsets[ci]
                x = x_sb[ci]

                h_sb = []
                for m in range(mh):
                    ps = ps1.tile([128, nl], fp32, tag="ps1", name=f"ps1_{ci}_{m}")
                    for k in range(_KC):
                        nc.tensor.matmul(
                            ps[:, :],
                            w1_sb[:, k, m * 128 : (m + 1) * 128],
                            x[:, k, :],
                            start=(k == 0),
                            stop=(k == _KC - 1),
                        )
                    ht = hpool.tile([128, nl], bf16, tag=f"h{m}", name=f"h_{ci}_{m}")
                    nc.scalar.activation(
                        ht[:, :], ps[:, :], act.Relu, bias=b_sb[:, m : m + 1]
                    )
                    h_sb.append(ht)

                yt = ypool.tile([128, md, nl], bf16, tag="y", name=f"y_{ci}")
                for d in range(md):
                    ps = ps2.tile([128, nl], fp32, tag="ps2", name=f"ps2_{ci}_{d}")
                    for m in range(mh):
                        nc.tensor.matmul(
                            ps[:, :],
                            w2_sb[:, m, d * 128 : (d + 1) * 128],
                            h_sb[m][:, :],
                            start=(m == 0),
                            stop=(m == mh - 1),
                        )
                    nc.vector.tensor_scalar_add(
                        yt[:, d, :], ps[:, :], b_sb[:, mh + d : mh + d + 1]
                    )
                nc.sync.dma_start(out=od[:, :, n0 : n0 + nl], in_=yt[:, :, :])

    nc.compile()
    return nc


def _get_bass(C: int):
    nc = _kernel_cache.get(C)
    if nc is None:
        nc = _build_bass(C)
        _kernel_cache[C] = nc
    return nc


def _prepare_in_maps(latents, actions, policy_indices, W1, b1, W2, b2):
    """Expert-parallel dispatch: returns (in_maps, C, order, offs, counts)."""
    latents = np.asarray(latents, dtype=np.float32)
    actions = np.asarray(actions, dtype=np.float32)
    pi = np.asarray(policy_indices).astype(np.int64)
    W1 = np.asarray(W1, dtype=np.float32)
    b1 = np.asarray(b1, dtype=np.float32)
    W2 = np.asarray(W2, dtype=np.float32)
    b2 = np.asarray(b2, dtype=np.float32)

    B = latents.shape[0]
    counts = np.bincount(pi, minlength=_P)
    order = np.argsort(pi, kind="stable")
    offs = np.concatenate(([0], np.cumsum(counts)))

    C = max(256, int(math.ceil(counts.max() / 128)) * 128)

    x = np.empty((B, _DA), dtype=np.float32)
    x[:, :_D] = latents
    x[:, _D:] = actions
    x_sorted = x[order].astype(_BF16)

    mh = _H // 128
    md = _D // 128
    in_maps = []
    for p in range(_P):
        xp = np.zeros((C, _KC * 128), dtype=_BF16)
        xp[: counts[p], :_DA] = x_sorted[offs[p] : offs[p + 1]]
        xq = np.ascontiguousarray(
            xp.T.reshape(_KC, 128, C).transpose(1, 0, 2)
        )
        w1p = np.zeros((_KC * 128, _H), dtype=np.float32)
        w1p[:_DA] = W1[p]
        w1q = np.ascontiguousarray(
            w1p.astype(_BF16).reshape(_KC, 128, _H).transpose(1, 0, 2)
        )
        w2q = np.ascontiguousarray(
            W2[p].astype(_BF16).reshape(mh, 128, _D).transpose(1, 0, 2)
        )
        bq = np.empty((128, mh + md), dtype=np.float32)
        bq[:, :mh] = b1[p].reshape(mh, 128).T
        bq[:, mh:] = b2[p].reshape(md, 128).T
        in_maps.append({"xq": xq, "w1q": w1q, "w2q": w2q, "bq": bq})
    return in_maps, C, order, offs, counts


def kernel(latents, actions, policy_indices, W1, b1, W2, b2):
    from concourse.bass_utils import run_bass_kernel_spmd

    in_maps, C, order, offs, counts = _prepare_in_maps(
        latents, actions, policy_indices, W1, b1, W2, b2
    )
    nc = _get_bass(C)
    results = run_bass_kernel_spmd(nc, in_maps, list(range(_N_CORES))).results

    B = np.asarray(latents).shape[0]
    out = np.empty((B, _D), dtype=np.float32)
    for p in range(_P):
        oq = np.asarray(results[p]["oq"])  # [128, 4, C] bf16
        yT = oq.transpose(1, 0, 2).reshape(_D, C)
        out[order[offs[p] : offs[p + 1]]] = yT[:, : counts[p]].T.astype(np.float32)
    return out


# revision 6
# speedup vs baseline: 1.0826x; 1.0696x over previous
"""MoE routed dynamics kernel for Trainium2 (8 NeuronCores, expert-parallel).

Problem: for each row b of a [B, D+A] input, route through one of P=8
two-layer MLPs selected by policy_indices[b]:
    h = relu(x @ W1[p] + b1[p]);  y = h @ W2[p] + b2[p]

Sharding: expert-parallel. Core p owns expert p's weights and processes
the rows routed to expert p. The all-to-all dispatch keyed on
policy_indices happens on the host at shard time (gather rows by expert,
pad to a common capacity C multiple of 128), and the inverse scatter at
unshard time.

Design notes (from trace analysis of three variants):
- Matmul dtype is float32r: measured steady-state pitch for an N=512
  matmul is 227ns (fp32r) vs 259ns (bf16) on this silicon — fp32r HIGH
  mode streams ~14% faster, and it needs no input quantization. fp32r
  requires N>=256 to stream at 1 cycle/col, so every column chunk is
  >=256 wide.
- The PE clock ramps for the first ~20us of the kernel regardless of
  activity; warmup matmuls on a zeroed tile burn that window during the
  initial DMA wait.
- DMA issue on Sync costs ~600ns each (the old baseline serialized 81
  issues = 51us). Host pre-arranges k-major layouts so each logical
  transfer is ONE descriptor-friendly DMA (~16 total), split across
  sync/scalar (HWDGE) for startup-critical loads and gpsimd (SWDGE,
  ~25ns dispatch) for bulk, ordered by first-use time.
- Output is stored bf16 and upcast on the host (error ~0.2%, halves
  store bytes and shortens the kernel tail).
"""

import math

import numpy as np
import ml_dtypes

_B = 16384
_P = 8
_D = 512
_A = 64
_H = 1024
_DA = _D + _A   # 576
_KC = 5         # K chunks over DA padded to 5*128=640
_N_CORES = 8

_kernel_cache: dict = {}


def _chunks(C: int):
    """Column chunking: 256-wide lead-in (halves the first x transfer,
    so the PE starts sooner), 512-wide steady chunks, and a tail split
    that keeps every chunk >=256 (fp32r needs N>=256 for full rate)."""
    assert C % 128 == 0 and C >= 256, C
    if C <= 512:
        return [C]
    out = [256]
    rem = C - 256
    while rem >= 1024:
        out.append(512)
        rem -= 512
    if rem <= 512:
        out.append(rem)
    elif rem - 512 >= 256:
        out += [512, rem - 512]
    else:
        out += [rem - 256, 256]
    return out


def _build_bass(C: int):
    import concourse.bacc as bacc
    import concourse.mybir as mybir
    from concourse.tile import TileContext

    fp32 = mybir.dt.float32
    f32r = mybir.dt.float32r
    bf16 = mybir.dt.bfloat16
    act = mybir.ActivationFunctionType

    widths = _chunks(C)
    offsets = [sum(widths[:i]) for i in range(len(widths))]
    mh = _H // 128  # 8 L1 output groups
    md = _D // 128  # 4 L2 output groups

    nc = bacc.Bacc()
    xd = nc.declare_dram_parameter("xq", [128, _KC, C], f32r, isOutput=False)
    w1d = nc.declare_dram_parameter("w1q", [128, _KC, _H], f32r, isOutput=False)
    w2d = nc.declare_dram_parameter("w2q", [128, mh, _D], f32r, isOutput=False)
    bd = nc.declare_dram_parameter("bq", [128, mh + md], fp32, isOutput=False)
    od = nc.declare_dram_parameter("oq", [128, md, C], bf16, isOutput=True)

    with TileContext(nc) as tc:
        with (
            tc.tile_pool(name="wpool", bufs=1) as wpool,
            tc.tile_pool(name="xpool", bufs=len(widths)) as xpool,
            tc.tile_pool(name="hpool", bufs=2) as hpool,
            tc.tile_pool(name="ypool", bufs=2) as ypool,
            tc.tile_pool(name="ps1", bufs=4, space="PSUM") as ps1,
            tc.tile_pool(name="ps2", bufs=4, space="PSUM") as ps2,
        ):
            w1_sb = wpool.tile([128, _KC, _H], f32r, tag="w1")
            w2_sb = wpool.tile([128, mh, _D], f32r, tag="w2")
            b_sb = wpool.tile([128, mh + md], fp32, tag="b")
            warm_sb = wpool.tile([128, 640], bf16, tag="warm")

            x_sb = [
                xpool.tile([128, _KC, nl], f32r, tag="x", name=f"x{ci}")
                for ci, nl in enumerate(widths)
            ]

            # Startup-critical loads on the two HWDGE engines (concurrent
            # with each other and with gpsimd SWDGE below), ordered by
            # first-use time on the PE.
            nc.sync.dma_start(out=x_sb[0][:, :, :], in_=xd[:, :, 0 : widths[0]])
            nc.scalar.dma_start(out=w1_sb[:, :, 0:128], in_=w1d[:, :, 0:128])
            nc.scalar.dma_start(out=b_sb[:, :], in_=bd[:, :])
            # Warmup source tile (zeros) for the PE clock ramp.
            nc.vector.memset(warm_sb[:, :], 0.0)
            # Bulk loads on gpsimd (software DGE: ~25ns engine dispatch,
            # ~1us generation each on the otherwise idle Pool engine),
            # in first-use order.
            nc.gpsimd.dma_start(out=w1_sb[:, :, 128:384], in_=w1d[:, :, 128:384])
            nc.gpsimd.dma_start(out=w1_sb[:, :, 384:768], in_=w1d[:, :, 384:768])
            nc.gpsimd.dma_start(out=w1_sb[:, :, 768:_H], in_=w1d[:, :, 768:_H])
            if len(widths) > 1:
                nc.gpsimd.dma_start(
                    out=x_sb[1][:, :, :],
                    in_=xd[:, :, offsets[1] : offsets[1] + widths[1]],
                )
            nc.gpsimd.dma_start(out=w2_sb[:, :, 0:256], in_=w2d[:, :, 0:256])
            nc.gpsimd.dma_start(out=w2_sb[:, :, 256:_D], in_=w2d[:, :, 256:_D])
            for ci in range(2, len(widths)):
                n0, nl = offsets[ci], widths[ci]
                nc.gpsimd.dma_start(
                    out=x_sb[ci][:, :, :], in_=xd[:, :, n0 : n0 + nl]
                )

            # PE warmup: the clock gate holds the PE at reduced rate for
            # the first ~20us; burn the DMA-wait window ramping it.
            for _ in range(6):
                wp = ps1.tile([128, 512], fp32, tag="ps1", name="warmps")
                nc.tensor.matmul(
                    wp[:, :], warm_sb[:, 0:128], warm_sb[:, 128:640],
                    start=True, stop=True,
                )

            for ci, nl in enumerate(widths):
                n0 = offsets[ci]
                x = x_sb[ci]

                h_sb = []
                for m in range(mh):
                    ps = ps1.tile([128, nl], fp32, tag="ps1", name=f"ps1_{ci}_{m}")
                    for k in range(_KC):
                        nc.tensor.matmul(
                            ps[:, :],
                            w1_sb[:, k, m * 128 : (m + 1) * 128],
                            x[:, k, :],
                            start=(k == 0),
                            stop=(k == _KC - 1),
                        )
                    ht = hpool.tile([128, nl], f32r, tag=f"h{m}", name=f"h_{ci}_{m}")
                    nc.scalar.activation(
                        ht[:, :], ps[:, :], act.Relu, bias=b_sb[:, m : m + 1]
                    )
                    h_sb.append(ht)

                yt = ypool.tile([128, md, nl], bf16, tag="y", name=f"y_{ci}")
                for d in range(md):
                    ps = ps2.tile([128, nl], fp32, tag="ps2", name=f"ps2_{ci}_{d}")
                    for m in range(mh):
                        nc.tensor.matmul(
                            ps[:, :],
                            w2_sb[:, m, d * 128 : (d + 1) * 128],
                            h_sb[m][:, :],
                            start=(m == 0),
                            stop=(m == mh - 1),
                        )
                    nc.vector.tensor_scalar_add(
                        yt[:, d, :], ps[:, :], b_sb[:, mh + d : mh + d + 1]
                    )
                nc.sync.dma_start(out=od[:, :, n0 : n0 + nl], in_=yt[:, :, :])

    nc.compile()
    return nc


def _get_bass(C: int):
    nc = _kernel_cache.get(C)
    if nc is None:
        nc = _build_bass(C)
        _kernel_cache[C] = nc
    return nc


def _prepare_in_maps(latents, actions, policy_indices, W1, b1, W2, b2):
    """Expert-parallel dispatch: returns (in_maps, C, order, offs, counts)."""
    latents = np.asarray(latents, dtype=np.float32)
    actions = np.asarray(actions, dtype=np.float32)
    pi = np.asarray(policy_indices).astype(np.int64)
    W1 = np.asarray(W1, dtype=np.float32)
    b1 = np.asarray(b1, dtype=np.float32)
    W2 = np.asarray(W2, dtype=np.float32)
    b2 = np.asarray(b2, dtype=np.float32)

    B = latents.shape[0]
    counts = np.bincount(pi, minlength=_P)
    order = np.argsort(pi, kind="stable")
    offs = np.concatenate(([0], np.cumsum(counts)))

    C = max(256, int(math.ceil(counts.max() / 128)) * 128)

    x = np.empty((B, _DA), dtype=np.float32)
    x[:, :_D] = latents
    x[:, _D:] = actions
    x_sorted = x[order]

    mh = _H // 128
    md = _D // 128
    in_maps = []
    for p in range(_P):
        xp = np.zeros((C, _KC * 128), dtype=np.float32)
        xp[: counts[p], :_DA] = x_sorted[offs[p] : offs[p + 1]]
        xq = np.ascontiguousarray(xp.T.reshape(_KC, 128, C).transpose(1, 0, 2))
        w1p = np.zeros((_KC * 128, _H), dtype=np.float32)
        w1p[:_DA] = W1[p]
        w1q = np.ascontiguousarray(w1p.reshape(_KC, 128, _H).transpose(1, 0, 2))
        w2q = np.ascontiguousarray(W2[p].reshape(mh, 128, _D).transpose(1, 0, 2))
        bq = np.empty((128, mh + md), dtype=np.float32)
        bq[:, :mh] = b1[p].reshape(mh, 128).T
        bq[:, mh:] = b2[p].reshape(md, 128).T
        in_maps.append({"xq": xq, "w1q": w1q, "w2q": w2q, "bq": bq})
    return in_maps, C, order, offs, counts


def kernel(latents, actions, policy_indices, W1, b1, W2, b2):
    from concourse.bass_utils import run_bass_kernel_spmd

    in_maps, C, order, offs, counts = _prepare_in_maps(
        latents, actions, policy_indices, W1, b1, W2, b2
    )
    nc = _get_bass(C)
    results = run_bass_kernel_spmd(nc, in_maps, list(range(_N_CORES))).results

    B = np.asarray(latents).shape[0]
    out = np.empty((B, _D), dtype=np.float32)
    for p in range(_P):
        oq = np.asarray(results[p]["oq"])  # [128, 4, C] bf16
        yT = oq.transpose(1, 0, 2).reshape(_D, C)
        out[order[offs[p] : offs[p + 1]]] = yT[:, : counts[p]].T.astype(np.float32)
    return out
